# revision 2
# baseline (speedup 1.0000x reference)
"""Deformable 3x3 conv (torchvision offset layout), N=8,C=O=256,H=W=64,
stride=1,pad=1,dil=1 on 8 NeuronCores, data-parallel over batch.

v2 pipeline (one image per core):
  - host stages layouts only (no math): xp pair-row table in DRAM
    (row (y,x) = [x[:,y,x] | x[:,y+1,x]] fp16, 512 vals), offsets
    permuted to the on-chip sample layout s = q*128 + p, weights
    pre-transposed to matmul-stationary layout, fp16.
  - device: offset -> floor/fraction/clamp -> idx table (int16, 16-part
    wrap, 8 Q7 replicas) + 4 per-slot corner weights (e-indicator
    shuffle handles all border clamps).
  - dma_gather: ONE 2KB descriptor per (tap, sample) fetches all 4
    corners ([A|B] = row idx, [C|D] = row idx+1).
  - DVE: 4-op chain applies corner weights (per-partition scalars).
  - PE: transpose to [c, s] then fp16 matmul, fp32 PSUM accumulated
    incrementally across all 9 taps; ACT evacuates with bias add.
  - per-region contiguous 1MB output DMA.
"""
import numpy as np

import concourse.bacc as bacc
import concourse.mybir as mybir
import concourse.tile as tile
from concourse import bass_utils
from concourse.bass import AP

F16 = mybir.dt.float16
F32 = mybir.dt.float32
I16 = mybir.dt.int16
I32 = mybir.dt.int32
ALU = mybir.AluOpType
ACTF = mybir.ActivationFunctionType

N, C, O, H, W, KS = 8, 256, 256, 64, 64, 3
K = KS * KS
S = H * W            # 4096 samples; sample s lives at (p = s % 128, q = s // 128)
QC = S // 128        # 32 q per partition
R = 8                # regions
QR = QC // R         # 4 q per region
KQ = K * QC          # 288

_CACHE = {}


def _build(bench_rep=None):
    nc = bacc.Bacc("TRN2", target_bir_lowering=False, debug=False,
                   enable_asserts=True, num_devices=8)
    xpd = nc.dram_tensor("xpd", [(S + 1) * 512], F16, kind="ExternalInput")
    offd = nc.dram_tensor("offd", [128, 2 * K * QC], F32, kind="ExternalInput")
    wtd = nc.dram_tensor("wtd", [128, 2 * K * 256], F16, kind="ExternalInput")
    bin_ = nc.dram_tensor("bin", [O], F32, kind="ExternalInput")
    byT = nc.dram_tensor("byT", [128, KQ], F32, kind="ExternalInput")
    bxT = nc.dram_tensor("bxT", [128, KQ], F32, kind="ExternalInput")
    id16 = nc.dram_tensor("id16", [128, 128], F16, kind="ExternalInput")
    id32 = nc.dram_tensor("id32", [128, 128], F32, kind="ExternalInput")
    out = nc.dram_tensor("out", [O, S], F32, kind="ExternalOutput")

    with tile.TileContext(nc) as tc:
        with tc.tile_pool(name="const", bufs=1) as cp, \
             tc.tile_pool(name="prep", bufs=1) as pp:
            i16 = cp.tile([128, 128], F16)
            nc.sync.dma_start(i16[:, :], id16[:, :])
            i32 = cp.tile([128, 128], F32)
            nc.sync.dma_start(i32[:, :], id32[:, :])
            bias_sb = cp.tile([128, 2], F32)
            nc.sync.dma_start(bias_sb[:, :], AP(bin_, 0, [[1, 128], [128, 2]]))
            wt = cp.tile([128, 2 * K, 256], F16)
            nc.sync.dma_start(
                wt[:, :, :],
                AP(wtd, 0, [[2 * K * 256, 128], [256, 2 * K], [1, 256]]))
            table = cp.tile([128, K * 256], I16)
            wA = cp.tile([128, KQ], F32)
            wB = cp.tile([128, KQ], F32)
            wC = cp.tile([128, KQ], F32)
            wD = cp.tile([128, KQ], F32)
            osb = cp.tile([128, 2, S], F32)

            def f32t(tag):
                return pp.tile([128, KQ], F32, tag=tag, name=tag)

            def ts2(dst, src, s1, s2, o1, o2):
                nc.vector.tensor_scalar(out=dst[:, :], in0=src[:, :], scalar1=s1,
                                        scalar2=s2, op0=o1, op1=o2)

            def tt(dst, a, b, op):
                nc.vector.tensor_tensor(dst[:, :], a[:, :], b[:, :], op)

            # ---- offsets (already in device sample order, s = q*128 + p)
            offy = pp.tile([128, K, QC], F32, tag="offy")
            nc.sync.dma_start(
                offy[:, :, :],
                AP(offd, 0, [[2 * KQ, 128], [2 * QC, K], [1, QC]]))
            offx = pp.tile([128, K, QC], F32, tag="offx")
            nc.sync.dma_start(
                offx[:, :, :],
                AP(offd, QC, [[2 * KQ, 128], [2 * QC, K], [1, QC]]))
            byt = f32t("byt")
            nc.sync.dma_start(byt[:, :], byT[:, :])
            bxt = f32t("bxt")
            nc.sync.dma_start(bxt[:, :], bxT[:, :])

            def floor8(pos8, tg):
                ii = pp.tile([128, KQ], I32, tag=tg + "i", name=tg + "i")
                nc.vector.tensor_copy(ii[:, :], pos8[:, :])
                rr = f32t(tg + "r")
                nc.vector.tensor_copy(rr[:, :], ii[:, :])
                mm = f32t(tg + "m")
                nc.vector.tensor_tensor(mm[:, :], rr[:, :], pos8[:, :], ALU.is_gt)
                ff = f32t(tg + "f")
                nc.vector.tensor_tensor(ff[:, :], rr[:, :], mm[:, :], ALU.subtract)
                return ff

            # --- indices first so gathers can start early
            py8 = f32t("py8")
            nc.vector.tensor_tensor(
                py8[:, :], offy[:, :, :].rearrange("P a b -> P (a b)"),
                byt[:, :], ALU.add)
            y0f = floor8(py8, "y0")
            y0c = f32t("y0c")
            ts2(y0c, y0f, 0.0, 62.0, ALU.max, ALU.min)

            px8 = f32t("px8")
            nc.vector.tensor_tensor(
                px8[:, :], offx[:, :, :].rearrange("P a b -> P (a b)"),
                bxt[:, :], ALU.add)
            x0f = floor8(px8, "x0")
            x0c = f32t("x0c")
            ts2(x0c, x0f, 0.0, 62.0, ALU.max, ALU.min)

            flS = pp.tile([128, K, QC], F32, tag="flS")
            nc.vector.scalar_tensor_tensor(
                out=flS[:, :, :].rearrange("P a b -> P (a b)"), in0=y0c[:, :],
                scalar=64.0, in1=x0c[:, :], op0=ALU.mult, op1=ALU.add)

            # ---- idx table: [16, k*256 + r*64 + ql*8 + ph] = flS[ph*16+p16, k, r*8+ql]
            with tc.tile_pool(name="psA", bufs=2, space="PSUM") as psA:
                for k in range(K):
                    pa = psA.tile([QC, 128], F32, tag="pa")
                    nc.tensor.transpose(pa[:, :], flS[:, k, :], i32[:, :])
                    asb = pp.tile([QC, 128], F32, tag="asb")
                    nc.vector.tensor_copy(asb[:, :], pa[:, :])
                    pt = psA.tile([16, 8, QC], F32, tag="pt")
                    for ph in range(8):
                        nc.tensor.transpose(
                            pt[:, ph, :], asb[:, ph * 16:(ph + 1) * 16],
                            i32[0:QC, 0:QC])
                    dst = table[0:16, k * 256:(k + 1) * 256].rearrange(
                        "P (r q ph) -> P ph r q", r=R, q=QR)
                    src = pt[:, :, :].rearrange("P a (r q) -> P a r q", r=R)
                    nc.vector.tensor_copy(dst, src)
            for g in range(1, 8):
                nc.sync.dma_start(table[g * 16:(g + 1) * 16, :], table[0:16, :])

            # ---- corner weights (overlap with first gathers)
            fy = f32t("fy")
            tt(fy, py8, y0f, ALU.subtract)
            fy1 = f32t("fy1")
            ts2(fy1, fy, -1.0, 1.0, ALU.mult, ALU.add)
            dyd = f32t("dyd")
            tt(dyd, y0f, y0c, ALU.subtract)
            eqs = []
            for tg, val in (("ey0", 0.0), ("eym", -1.0), ("eyp", 1.0)):
                e = f32t(tg)
                nc.vector.tensor_scalar(out=e[:, :], in0=dyd[:, :], scalar1=val,
                                        scalar2=None, op0=ALU.is_equal)
                eqs.append(e)
            ey0, eym, eyp = eqs
            wyA = f32t("wyA")
            t1 = f32t("wy1")
            tt(t1, fy1, ey0, ALU.mult)
            t2 = f32t("wy2")
            tt(t2, fy, eym, ALU.mult)
            tt(wyA, t1, t2, ALU.add)
            wyB = f32t("wyB")
            t3 = f32t("wy3")
            tt(t3, fy, ey0, ALU.mult)
            t4 = f32t("wy4")
            tt(t4, fy1, eyp, ALU.mult)
            tt(wyB, t3, t4, ALU.add)

            fx = f32t("fx")
            tt(fx, px8, x0f, ALU.subtract)
            fx1 = f32t("fx1")
            ts2(fx1, fx, -1.0, 1.0, ALU.mult, ALU.add)
            dxd = f32t("dxd")
            tt(dxd, x0f, x0c, ALU.subtract)
            eqs = []
            for tg, val in (("ex0", 0.0), ("exm", -1.0), ("exp", 1.0)):
                e = f32t(tg)
                nc.vector.tensor_scalar(out=e[:, :], in0=dxd[:, :], scalar1=val,
                                        scalar2=None, op0=ALU.is_equal)
                eqs.append(e)
            ex0, exm, exp_ = eqs
            wxA = f32t("wxA")
            u1 = f32t("wx1")
            tt(u1, fx1, ex0, ALU.mult)
            u2 = f32t("wx2")
            tt(u2, fx, exm, ALU.mult)
            tt(wxA, u1, u2, ALU.add)
            wxB = f32t("wxB")
            u3 = f32t("wx3")
            tt(u3, fx, ex0, ALU.mult)
            u4 = f32t("wx4")
            tt(u4, fx1, exp_, ALU.mult)
            tt(wxB, u3, u4, ALU.add)

            tt(wA, wyA, wxA, ALU.mult)
            tt(wB, wyB, wxA, ALU.mult)
            tt(wC, wyA, wxB, ALU.mult)
            tt(wD, wyB, wxB, ALU.mult)

            # ================= main loop =================
            in_ap = AP(xpd, 0, [[512, S], [1, 1024]])
            with tc.tile_pool(name="gpool", bufs=3) as gp, \
                 tc.tile_pool(name="spool", bufs=2) as sp, \
                 tc.tile_pool(name="tmp", bufs=4) as mp, \
                 tc.tile_pool(name="psT", bufs=2, space="PSUM") as psT, \
                 tc.tile_pool(name="psC", bufs=2, space="PSUM") as psC:
                for _rep in range(bench_rep or 1):
                    _main(nc, tc, gp, sp, mp, psT, psC, table, wt, wA, wB, wC,
                          wD, bias_sb, osb, i16, in_ap, out)

    nc.compile()
    return nc


def _main(nc, tc, gp, sp, mp, psT, psC, table, wt, wA, wB, wC, wD, bias_sb,
          osb, i16, in_ap, out):
    if True:
            if True:
                for r in range(R):
                    pc = psC.tile([128, 2, 512], F32, tag="pc")
                    for k in range(K):
                        g = gp.tile([128, QR, 1024], F16, tag="g")
                        nc.gpsimd.dma_gather(
                            out_ap=g[:, :, :], in_ap=in_ap,
                            idxs_ap=table[:, k * 256 + r * 32:k * 256 + r * 32 + 32],
                            num_idxs=128 * QR, num_idxs_reg=128 * QR,
                            elem_size=1024, elem_step=512,
                            single_packet=False)
                        ptt = psT.tile([128, 2, QR, 128], F32, tag="ptt")
                        for ql in range(QR):
                            col = k * QC + r * QR + ql
                            # paired scale tiles: sc1 = [As|Cs], sc2 = [Bs|Ds]
                            # A,B,D on DVE (TS 4x), C on ACT; one TT add gives
                            # [t01|t23].
                            sc1 = mp.tile([128, 512], F16, tag="sc1")
                            nc.vector.tensor_scalar(
                                out=sc1[:, 0:256], in0=g[:, ql, 0:256],
                                scalar1=wA[:, col:col + 1], scalar2=None,
                                op0=ALU.mult)
                            nc.scalar.mul(sc1[:, 256:512], g[:, ql, 512:768],
                                          wC[:, col:col + 1])
                            sc2 = mp.tile([128, 512], F16, tag="sc2")
                            nc.vector.tensor_scalar(
                                out=sc2[:, 0:256], in0=g[:, ql, 256:512],
                                scalar1=wB[:, col:col + 1], scalar2=None,
                                op0=ALU.mult)
                            nc.vector.tensor_scalar(
                                out=sc2[:, 256:512], in0=g[:, ql, 768:1024],
                                scalar1=wD[:, col:col + 1], scalar2=None,
                                op0=ALU.mult)
                            tq = mp.tile([128, 512], F16, tag="tq")
                            nc.vector.tensor_tensor(
                                tq[:, :], sc1[:, :], sc2[:, :], ALU.add)
                            # transpose the two partials, summing in f32 PSUM
                            for ch in range(2):
                                nc.tensor.matmul(
                                    ptt[:, ch, ql, :],
                                    tq[:, ch * 128:(ch + 1) * 128], i16[:, :],
                                    start=True, stop=False)
                                nc.tensor.matmul(
                                    ptt[:, ch, ql, :],
                                    tq[:, 256 + ch * 128:256 + (ch + 1) * 128],
                                    i16[:, :],
                                    start=False, stop=True)
                        sam = sp.tile([128, 2, QR * 128], F16, tag="sam")
                        nc.scalar.copy(
                            sam[:, :, :].rearrange("P a b -> P (a b)"),
                            ptt[:, :, :, :].rearrange("P a b c -> P (a b c)"))
                        for chc in range(2):
                            for och in range(2):
                                nc.tensor.matmul(
                                    pc[:, och, :],
                                    wt[:, 2 * k + chc, och * 128:(och + 1) * 128],
                                    sam[:, chc, :],
                                    start=(k == 0 and chc == 0),
                                    stop=(k == K - 1 and chc == 1))
                    for och in range(2):
                        nc.scalar.activation(
                            out=osb[:, och, r * 512:(r + 1) * 512],
                            in_=pc[:, och, :],
                            func=ACTF.Identity,
                            bias=bias_sb[:, och:och + 1], scale=1.0)
                    nc.sync.dma_start(
                        AP(out, r * 512, [[S, 128], [128 * S, 2], [1, 512]]),
                        osb[:, :, r * 512:(r + 1) * 512])

    nc.compile()
    return nc


def _consts():
    p = np.arange(128)[:, None].astype(np.float32)
    q = np.arange(QC)[None, :].astype(np.float32)
    by = np.zeros((128, KQ), np.float32)
    bx = np.zeros((128, KQ), np.float32)
    for k in range(K):
        by[:, k * QC:(k + 1) * QC] = 2.0 * q + np.floor(p / 64) - 1.0 + (k // KS)
        bx[:, k * QC:(k + 1) * QC] = np.mod(p, 64) - 1.0 + (k % KS)
    return by, bx


def _smap():
    # flat row-major spatial index for device sample s = q*128 + p
    s = np.arange(S)
    p = s % 128
    q = s // 128
    i = 2 * q + p // 64
    j = p % 64
    return (i * 64 + j).astype(np.int64)  # [S]


def _in_maps(x, offset, weight, bias):
    by, bx = _consts()
    smf = _smap()
    id16 = np.eye(128, dtype=np.float16)
    id32 = np.eye(128, dtype=np.float32)
    x16 = np.asarray(x, np.float32).astype(np.float16)      # [N,C,H,W]
    xs = x16.transpose(0, 2, 3, 1)                          # [N,H,W,C]
    xp = np.zeros((N, S + 1, 512), np.float16)
    xp[:, :S].reshape(N, H, W, 512)[:, :, :, :256] = xs
    xp[:, :S].reshape(N, H, W, 512)[:, :63, :, 256:] = xs[:, 1:64]
    xp = np.ascontiguousarray(xp.reshape(N, (S + 1) * 512))
    off_f = np.asarray(offset, np.float32).reshape(N, 2 * K, S)
    offd = off_f[:, :, smf].transpose(0, 2, 1)              # [N,128*32? ->]
    # offd currently [N, 2K, S->smap applied] -> need [N, p, 2K, q]
    off_s = off_f[:, :, smf]                                # [N, 2K, S(dev order)]
    off_s = off_s.reshape(N, 2 * K, QC, 128)                # s = q*128 + p
    offd = np.ascontiguousarray(
        off_s.transpose(0, 3, 1, 2).reshape(N, 128, 2 * K * QC)).astype(np.float32)
    W9 = np.asarray(weight, np.float32).reshape(O, 2, 128, K)
    wtd = np.ascontiguousarray(
        W9.transpose(2, 3, 1, 0).reshape(128, 2 * K * 256)).astype(np.float16)
    bf = np.ascontiguousarray(np.asarray(bias, np.float32).reshape(O))
    return [{"xpd": xp[i], "offd": offd[i], "wtd": wtd, "bin": bf,
             "byT": by, "bxT": bx, "id16": id16, "id32": id32}
            for i in range(N)]


def _unmap_out(out_flat):
    # out_flat [O, S(dev order)] -> [O, H, W] row-major
    smf = _smap()
    img = np.empty((O, S), np.float32)
    img[:, smf] = out_flat
    return img.reshape(O, H, W)


def kernel(x, offset, weight, bias, stride):
    stride = int(np.asarray(stride))
    assert stride == 1, "only stride=1 supported"
    if "nc" not in _CACHE:
        _CACHE["nc"] = _build()
    nc = _CACHE["nc"]
    res = bass_utils.run_bass_kernel_spmd(nc, _in_maps(x, offset, weight, bias),
                                          core_ids=list(range(8)))
    outs = np.stack([_unmap_out(res.results[i]["out"]) for i in range(N)])
    return outs.astype(np.float32)
